# revision 10
# baseline (speedup 1.0000x reference)
"""Locally-connected conv (BioConvolution) Trainium2 kernel.

Problem: Z[n,p,o] = relu(sum_{ijc} patch[n,p,i,j,c] * filt[p,i,j,c,o] + bias[o])
  X: (32,128,128,32) f32, filters: (1024,4,4,32,32) f32, bias: (32,)
  out: (32,32,32,32) f32.   FH=FW=4 non-overlapping patches, P=1024.

Sharding: patch-parallel over P across 8 cores. Core k owns patches
[128k,128k+128) == image rows [16k,16k+16). Each core touches only its own
X rows and filters: 16.8 MB in + 0.5 MB out per core — the true memory
roofline (no operand is reused across cores).

Host-side marshaling (part of sharding): the contraction axis must sit on
SBUF partitions for the PE, so X is pre-arranged per-core into
  xt[r, p, q, b] = X[b, 16k+4*pr+q, 4*pc+j, c]   (r = j*32+c, p = pr*32+pc)
and filters into the matching
  ft[r, p, q, o] = filters[128k+p, q, j, c, o].
Both are r-major so every HBM->SBUF DMA moves long contiguous runs per
partition (multi-KB descriptors at line rate).

Device kernel (identical SPMD program on 8 cores):
  for each 4-patch group: 16 fp32 matmuls (K=128, M=32 fout, N=32 batch)
  accumulate into one PSUM [128,32] tile (col-tiled: patch s -> partitions
  32s..32s+32), ScalarE applies bias+ReLU into an SBUF staging buffer,
  one contiguous 512 KB output DMA at the end.
"""

import numpy as np

N, H, W, C = 32, 128, 128, 32
FH = FW = 4
FOUT = 32
NCORES = 8
PL = 128          # patches per core
NQ = 4            # K-chunks per patch (512 / 128)
KR = 128          # contraction rows per chunk (SBUF partitions)
NG = PL // 4      # 4-patch groups per core

_CACHE = {}


def _build_module(bufs=3, out_splits=8, mm_dtype="float32"):
    from concourse import bacc, tile, mybir

    nc = bacc.Bacc("TRN2", target_bir_lowering=False, debug=False)
    dt = mybir.dt.float32
    mdt = getattr(mybir.dt, mm_dtype)
    # xf packs data and filters: [..., 0:32] = batch cols, [..., 32:64] = fout
    xf = nc.dram_tensor("xf", [KR, PL, NQ, N + FOUT], mdt, kind="ExternalInput").ap()
    bt = nc.dram_tensor("bt", [KR, 1], dt, kind="ExternalInput").ap()
    out = nc.dram_tensor("out", [KR, NG, N], dt, kind="ExternalOutput").ap()

    # Graduated chunk sizes (in patches): small first chunks so the first
    # matmul isn't gated on a full-size load sharing bandwidth round-robin.
    sizes = [2, 2, 4]
    rest = PL - sum(sizes)
    sizes += [8] * (rest // 8)
    assert sum(sizes) == PL
    GSPLIT = NG // out_splits
    relu = mybir.ActivationFunctionType.Relu

    with tile.TileContext(nc) as tc:
        with (
            tc.tile_pool(name="xfpool", bufs=bufs) as xfpool,
            tc.tile_pool(name="psum", bufs=8, space="PSUM") as psum,
            tc.tile_pool(name="misc", bufs=1) as misc,
        ):
            bias_t = misc.tile([KR, 1], dt)
            nc.sync.dma_start(bias_t[:], bt[:])
            staging = misc.tile([KR, NG, N], dt)

            p0 = 0
            for ch, PC in enumerate(sizes):
                xtile = xfpool.tile([KR, PC, NQ, N + FOUT], mdt, tag="xf")
                sl = slice(p0, p0 + PC)
                eng = nc.sync if ch % 2 == 0 else nc.scalar
                eng.dma_start(xtile[:], xf[:, sl, :, :])
                for g in range(PC // 2):
                    gg = (p0 + g * 2) // 4       # psum group id (2 patches/iter)
                    half = (p0 + g * 2) % 4      # 0 or 2: which half of the group
                    if half == 0:
                        ptile = psum.tile([KR, N], dt, tag="ps")
                    for s2 in range(2):
                        s = half + s2
                        p = g * 2 + s2
                        for q in range(NQ):
                            nc.tensor.matmul(
                                ptile[32 * s : 32 * s + 32, :],
                                xtile[:, p, q, N : N + FOUT],  # lhsT [128,32(o)]
                                xtile[:, p, q, 0:N],           # rhs  [128,32(b)]
                                start=(q == 0),
                                stop=(q == NQ - 1),
                                tile_position=(0, 32 * s),
                            )
                    if half == 2:
                        nc.scalar.activation(
                            staging[:, gg, :], ptile[:], relu, bias=bias_t[:]
                        )
                        if (gg + 1) % GSPLIT == 0:
                            osl = slice(gg + 1 - GSPLIT, gg + 1)
                            nc.gpsimd.dma_start(out[:, osl, :], staging[:, osl, :])
                p0 += PC
    nc.compile()
    return nc


def _build_module_r(bufs=3, out_splits=8):
    """float32r variant: single-pass fp32 matmuls (tf32-ish), PSUM packing
    along the free axis (8 patches per bank) since fp32r requires dst
    base partition 0."""
    from concourse import bacc, tile, mybir

    nc = bacc.Bacc("TRN2", target_bir_lowering=False, debug=False)
    dt = mybir.dt.float32
    mdt = mybir.dt.float32r
    SG = 8                      # patches per PSUM super-group
    NSG = PL // SG              # 16
    xf = nc.dram_tensor("xf", [KR, PL, NQ, N + FOUT], mdt, kind="ExternalInput").ap()
    bt = nc.dram_tensor("bt", [FOUT, 1], dt, kind="ExternalInput").ap()
    out = nc.dram_tensor("out", [FOUT, PL, N], dt, kind="ExternalOutput").ap()

    sizes = [2, 2, 4]
    rest = PL - sum(sizes)
    sizes += [8] * (rest // 8)
    assert sum(sizes) == PL
    OSPLIT = NSG // out_splits
    relu = mybir.ActivationFunctionType.Relu

    with tile.TileContext(nc) as tc:
        with (
            tc.tile_pool(name="xfpool", bufs=bufs) as xfpool,
            tc.tile_pool(name="psum", bufs=6, space="PSUM") as psum,
            tc.tile_pool(name="misc", bufs=1) as misc,
        ):
            bias_t = misc.tile([FOUT, 1], dt)
            nc.sync.dma_start(bias_t[:], bt[:])
            staging = misc.tile([FOUT, NSG, SG, N], dt)

            p0 = 0
            ptile = None
            for ch, PC in enumerate(sizes):
                xtile = xfpool.tile([KR, PC, NQ, N + FOUT], mdt, tag="xf")
                eng = nc.sync if ch % 2 == 0 else nc.scalar
                eng.dma_start(xtile[:], xf[:, p0 : p0 + PC, :, :])
                for pl in range(PC):
                    p = p0 + pl
                    sg, i = divmod(p, SG)
                    if i == 0:
                        ptile = psum.tile([FOUT, SG, N], dt, tag="ps")
                    for q in range(NQ):
                        nc.tensor.matmul(
                            ptile[:, i, :],
                            xtile[:, pl, q, N : N + FOUT],  # lhsT [128,32(o)]
                            xtile[:, pl, q, 0:N],           # rhs  [128,32(b)]
                            start=(q == 0),
                            stop=(q == NQ - 1),
                        )
                    if i == SG - 1:
                        nc.scalar.activation(
                            staging[:, sg, :, :], ptile[:], relu, bias=bias_t[:]
                        )
                        if (sg + 1) % OSPLIT == 0:
                            osl = slice((sg + 1 - OSPLIT) * SG, (sg + 1) * SG)
                            nc.gpsimd.dma_start(out[:, osl, :], staging[:, (sg + 1 - OSPLIT) : sg + 1, :, :])
                p0 += PC
    nc.compile()
    return nc


def _get_module():
    if "nc" not in _CACHE:
        _CACHE["nc"] = _build_module()
    return _CACHE["nc"]


def _marshal(X, filters, bias):
    """Shard + lay out full inputs into per-core device arrays."""
    X = np.ascontiguousarray(np.asarray(X, dtype=np.float32))
    filters = np.ascontiguousarray(np.asarray(filters, dtype=np.float32))
    bias = np.asarray(bias, dtype=np.float32)

    # X: (b, core, pr, i, pc, j, c) -> (core, j, c, pr, pc, i, b)
    xv = X.reshape(N, NCORES, 4, FH, 32, FW, C)
    xt = xv.transpose(1, 5, 6, 2, 4, 3, 0).reshape(NCORES, KR, PL, NQ, N)
    # filters: (core, p, i, j, c, o) -> (core, j, c, p, i, o)
    fv = filters.reshape(NCORES, PL, FH, FW, C, FOUT)
    ft = fv.transpose(0, 3, 4, 1, 2, 5).reshape(NCORES, KR, PL, NQ, FOUT)
    xfa = np.ascontiguousarray(np.concatenate([xt, ft], axis=4))
    bt = np.ascontiguousarray(np.tile(bias, 4).reshape(KR, 1))
    return xfa, bt


def _assemble(outs):
    """Per-core out [128=(s,o), NG, N] -> full (N, 32, 32, FOUT)."""
    z = np.stack(outs)                                  # (core, (s,o), g, b)
    z = z.reshape(NCORES, 4, FOUT, NG, N)               # (core, s, o, g, b)
    z = z.transpose(4, 0, 3, 1, 2)                      # (b, core, g, s, o)
    z = z.reshape(N, NCORES, PL, FOUT)                  # p_loc = 4*g + s
    z = z.reshape(N, NCORES * 4, 32, FOUT)              # (b, pr_glob, pc, o)
    return np.ascontiguousarray(z)


def _assemble_r(outs):
    """Per-core out [FOUT, PL, N] -> full (N, 32, 32, FOUT)."""
    z = np.stack(outs)                                  # (core, o, p, b)
    z = z.transpose(3, 0, 2, 1)                         # (b, core, p, o)
    return np.ascontiguousarray(z.reshape(N, 32, 32, FOUT))


LAST_RESULT = None
VARIANT = "fp32r"


def kernel(X, filters, bias):
    global LAST_RESULT
    from concourse.bass_utils import run_bass_kernel_spmd

    if "nc" not in _CACHE:
        _CACHE["nc"] = (
            _build_module_r() if VARIANT == "fp32r" else _build_module()
        )
    nc = _CACHE["nc"]
    xfa, bt = _marshal(X, filters, bias)
    if VARIANT == "fp32r":
        bt = np.ascontiguousarray(bt[:FOUT])
    in_maps = [{"xf": xfa[k], "bt": bt} for k in range(NCORES)]
    res = run_bass_kernel_spmd(nc, in_maps, core_ids=list(range(NCORES)))
    LAST_RESULT = res
    outs = [res.results[k]["out"] for k in range(NCORES)]
    return _assemble_r(outs) if VARIANT == "fp32r" else _assemble(outs)


# revision 13
# speedup vs baseline: 1.0382x; 1.0382x over previous
"""Locally-connected conv (BioConvolution) Trainium2 kernel.

Problem: Z[n,p,o] = relu(sum_{ijc} patch[n,p,i,j,c] * filt[p,i,j,c,o] + bias[o])
  X: (32,128,128,32) f32, filters: (1024,4,4,32,32) f32, bias: (32,)
  out: (32,32,32,32) f32.   FH=FW=4 non-overlapping patches, P=1024.

Sharding: patch-parallel over P across 8 cores. Core k owns patches
[128k,128k+128) == image rows [16k,16k+16). Each core touches only its own
X rows and filters: 16.8 MB in + 0.5 MB out per core — the true memory
roofline (no operand is reused across cores).

Host-side marshaling (part of sharding): the contraction axis must sit on
SBUF partitions for the PE, so X is pre-arranged per-core into
  xt[r, p, q, b] = X[b, 16k+4*pr+q, 4*pc+j, c]   (r = j*32+c, p = pr*32+pc)
and filters into the matching
  ft[r, p, q, o] = filters[128k+p, q, j, c, o].
Both are r-major so every HBM->SBUF DMA moves long contiguous runs per
partition (multi-KB descriptors at line rate).

Device kernel (identical SPMD program on 8 cores):
  for each 4-patch group: 16 fp32 matmuls (K=128, M=32 fout, N=32 batch)
  accumulate into one PSUM [128,32] tile (col-tiled: patch s -> partitions
  32s..32s+32), ScalarE applies bias+ReLU into an SBUF staging buffer,
  one contiguous 512 KB output DMA at the end.
"""

import numpy as np

N, H, W, C = 32, 128, 128, 32
FH = FW = 4
FOUT = 32
NCORES = 8
PL = 128          # patches per core
NQ = 4            # K-chunks per patch (512 / 128)
KR = 128          # contraction rows per chunk (SBUF partitions)
NG = PL // 4      # 4-patch groups per core

_CACHE = {}


def _build_module(bufs=6, out_splits=8, mm_dtype="float32"):
    from concourse import bacc, tile, mybir

    nc = bacc.Bacc("TRN2", target_bir_lowering=False, debug=False)
    dt = mybir.dt.float32
    mdt = getattr(mybir.dt, mm_dtype)
    # xf packs data and filters: [..., 0:32] = batch cols, [..., 32:64] = fout
    xf = nc.dram_tensor("xf", [KR, PL, NQ, N + FOUT], mdt, kind="ExternalInput").ap()
    bt = nc.dram_tensor("bt", [KR, 1], dt, kind="ExternalInput").ap()
    out = nc.dram_tensor("out", [KR, NG, N], dt, kind="ExternalOutput").ap()

    # Graduated chunk sizes (in patches): small first chunks so the first
    # matmul isn't gated on a full-size load sharing bandwidth round-robin.
    sizes = [2, 2, 4]
    rest = PL - sum(sizes)
    sizes += [8] * (rest // 8)
    assert sum(sizes) == PL
    GSPLIT = NG // out_splits
    relu = mybir.ActivationFunctionType.Relu

    with tile.TileContext(nc) as tc:
        with (
            tc.tile_pool(name="xfpool", bufs=bufs) as xfpool,
            tc.tile_pool(name="psum", bufs=8, space="PSUM") as psum,
            tc.tile_pool(name="misc", bufs=1) as misc,
        ):
            bias_t = misc.tile([KR, 1], dt)
            nc.sync.dma_start(bias_t[:], bt[:])
            staging = misc.tile([KR, NG, N], dt)

            p0 = 0
            for ch, PC in enumerate(sizes):
                xtile = xfpool.tile([KR, PC, NQ, N + FOUT], mdt, tag="xf")
                sl = slice(p0, p0 + PC)
                eng = nc.sync if ch % 2 == 0 else nc.scalar
                eng.dma_start(xtile[:], xf[:, sl, :, :])
                for g in range(PC // 2):
                    gg = (p0 + g * 2) // 4       # psum group id (2 patches/iter)
                    half = (p0 + g * 2) % 4      # 0 or 2: which half of the group
                    if half == 0:
                        ptile = psum.tile([KR, N], dt, tag="ps")
                    for s2 in range(2):
                        s = half + s2
                        p = g * 2 + s2
                        for q in range(NQ):
                            nc.tensor.matmul(
                                ptile[32 * s : 32 * s + 32, :],
                                xtile[:, p, q, N : N + FOUT],  # lhsT [128,32(o)]
                                xtile[:, p, q, 0:N],           # rhs  [128,32(b)]
                                start=(q == 0),
                                stop=(q == NQ - 1),
                                tile_position=(0, 32 * s),
                            )
                    if half == 2:
                        nc.scalar.activation(
                            staging[:, gg, :], ptile[:], relu, bias=bias_t[:]
                        )
                        if (gg + 1) % GSPLIT == 0:
                            osl = slice(gg + 1 - GSPLIT, gg + 1)
                            oeng = nc.sync if gg + 1 == NG else nc.gpsimd
                            oeng.dma_start(out[:, osl, :], staging[:, osl, :])
                p0 += PC
    nc.compile()
    return nc


def _build_module_r(bufs=3, out_splits=8):
    """float32r variant: single-pass fp32 matmuls (tf32-ish), PSUM packing
    along the free axis (8 patches per bank) since fp32r requires dst
    base partition 0."""
    from concourse import bacc, tile, mybir

    nc = bacc.Bacc("TRN2", target_bir_lowering=False, debug=False)
    dt = mybir.dt.float32
    mdt = mybir.dt.float32r
    SG = 8                      # patches per PSUM super-group
    NSG = PL // SG              # 16
    xf = nc.dram_tensor("xf", [KR, PL, NQ, N + FOUT], mdt, kind="ExternalInput").ap()
    bt = nc.dram_tensor("bt", [FOUT, 1], dt, kind="ExternalInput").ap()
    out = nc.dram_tensor("out", [FOUT, PL, N], dt, kind="ExternalOutput").ap()

    sizes = [2, 2, 4]
    rest = PL - sum(sizes)
    sizes += [8] * (rest // 8)
    assert sum(sizes) == PL
    OSPLIT = NSG // out_splits
    relu = mybir.ActivationFunctionType.Relu

    with tile.TileContext(nc) as tc:
        with (
            tc.tile_pool(name="xfpool", bufs=bufs) as xfpool,
            tc.tile_pool(name="psum", bufs=6, space="PSUM") as psum,
            tc.tile_pool(name="misc", bufs=1) as misc,
        ):
            bias_t = misc.tile([FOUT, 1], dt)
            nc.sync.dma_start(bias_t[:], bt[:])
            staging = misc.tile([FOUT, NSG, SG, N], dt)

            p0 = 0
            ptile = None
            for ch, PC in enumerate(sizes):
                xtile = xfpool.tile([KR, PC, NQ, N + FOUT], mdt, tag="xf")
                eng = nc.sync if ch % 2 == 0 else nc.scalar
                eng.dma_start(xtile[:], xf[:, p0 : p0 + PC, :, :])
                for pl in range(PC):
                    p = p0 + pl
                    sg, i = divmod(p, SG)
                    if i == 0:
                        ptile = psum.tile([FOUT, SG, N], dt, tag="ps")
                    for q in range(NQ):
                        nc.tensor.matmul(
                            ptile[:, i, :],
                            xtile[:, pl, q, N : N + FOUT],  # lhsT [128,32(o)]
                            xtile[:, pl, q, 0:N],           # rhs  [128,32(b)]
                            start=(q == 0),
                            stop=(q == NQ - 1),
                        )
                    if i == SG - 1:
                        nc.scalar.activation(
                            staging[:, sg, :, :], ptile[:], relu, bias=bias_t[:]
                        )
                        if (sg + 1) % OSPLIT == 0:
                            osl = slice((sg + 1 - OSPLIT) * SG, (sg + 1) * SG)
                            nc.gpsimd.dma_start(out[:, osl, :], staging[:, (sg + 1 - OSPLIT) : sg + 1, :, :])
                p0 += PC
    nc.compile()
    return nc


def _get_module():
    if "nc" not in _CACHE:
        _CACHE["nc"] = _build_module()
    return _CACHE["nc"]


def _marshal(X, filters, bias):
    """Shard + lay out full inputs into per-core device arrays."""
    X = np.ascontiguousarray(np.asarray(X, dtype=np.float32))
    filters = np.ascontiguousarray(np.asarray(filters, dtype=np.float32))
    bias = np.asarray(bias, dtype=np.float32)

    # X: (b, core, pr, i, pc, j, c) -> (core, j, c, pr, pc, i, b)
    xv = X.reshape(N, NCORES, 4, FH, 32, FW, C)
    xt = xv.transpose(1, 5, 6, 2, 4, 3, 0).reshape(NCORES, KR, PL, NQ, N)
    # filters: (core, p, i, j, c, o) -> (core, j, c, p, i, o)
    fv = filters.reshape(NCORES, PL, FH, FW, C, FOUT)
    ft = fv.transpose(0, 3, 4, 1, 2, 5).reshape(NCORES, KR, PL, NQ, FOUT)
    xfa = np.ascontiguousarray(np.concatenate([xt, ft], axis=4))
    bt = np.ascontiguousarray(np.tile(bias, 4).reshape(KR, 1))
    return xfa, bt


def _assemble(outs):
    """Per-core out [128=(s,o), NG, N] -> full (N, 32, 32, FOUT)."""
    z = np.stack(outs)                                  # (core, (s,o), g, b)
    z = z.reshape(NCORES, 4, FOUT, NG, N)               # (core, s, o, g, b)
    z = z.transpose(4, 0, 3, 1, 2)                      # (b, core, g, s, o)
    z = z.reshape(N, NCORES, PL, FOUT)                  # p_loc = 4*g + s
    z = z.reshape(N, NCORES * 4, 32, FOUT)              # (b, pr_glob, pc, o)
    return np.ascontiguousarray(z)


def _assemble_r(outs):
    """Per-core out [FOUT, PL, N] -> full (N, 32, 32, FOUT)."""
    z = np.stack(outs)                                  # (core, o, p, b)
    z = z.transpose(3, 0, 2, 1)                         # (b, core, p, o)
    return np.ascontiguousarray(z.reshape(N, 32, 32, FOUT))


LAST_RESULT = None
VARIANT = "fp32"


def kernel(X, filters, bias):
    global LAST_RESULT
    from concourse.bass_utils import run_bass_kernel_spmd

    if "nc" not in _CACHE:
        _CACHE["nc"] = (
            _build_module_r() if VARIANT == "fp32r" else _build_module()
        )
    nc = _CACHE["nc"]
    xfa, bt = _marshal(X, filters, bias)
    if VARIANT == "fp32r":
        bt = np.ascontiguousarray(bt[:FOUT])
    in_maps = [{"xf": xfa[k], "bt": bt} for k in range(NCORES)]
    res = run_bass_kernel_spmd(nc, in_maps, core_ids=list(range(NCORES)))
    LAST_RESULT = res
    outs = [res.results[k]["out"] for k in range(NCORES)]
    return _assemble_r(outs) if VARIANT == "fp32r" else _assemble(outs)


# revision 15
# speedup vs baseline: 1.1106x; 1.0697x over previous
"""Locally-connected conv (BioConvolution) Trainium2 kernel.

Problem: Z[n,p,o] = relu(sum_{ijc} patch[n,p,i,j,c] * filt[p,i,j,c,o] + bias[o])
  X: (32,128,128,32) f32, filters: (1024,4,4,32,32) f32, bias: (32,)
  out: (32,32,32,32) f32.   FH=FW=4 non-overlapping patches, P=1024.

Sharding: patch-parallel over P across 8 cores. Core k owns patches
[128k,128k+128) == image rows [16k,16k+16). Each core touches only its own
X rows and filters: 16.8 MB in + 0.5 MB out per core — the true memory
roofline (no operand is reused across cores).

Host-side marshaling (part of sharding): the contraction axis must sit on
SBUF partitions for the PE, so X is pre-arranged per-core into
  xt[r, p, q, b] = X[b, 16k+4*pr+q, 4*pc+j, c]   (r = j*32+c, p = pr*32+pc)
and filters into the matching
  ft[r, p, q, o] = filters[128k+p, q, j, c, o].
Both are r-major so every HBM->SBUF DMA moves long contiguous runs per
partition (multi-KB descriptors at line rate).

Device kernel (identical SPMD program on 8 cores):
  for each 4-patch group: 16 fp32 matmuls (K=128, M=32 fout, N=32 batch)
  accumulate into one PSUM [128,32] tile (col-tiled: patch s -> partitions
  32s..32s+32), ScalarE applies bias+ReLU into an SBUF staging buffer,
  one contiguous 512 KB output DMA at the end.
"""

import numpy as np

N, H, W, C = 32, 128, 128, 32
FH = FW = 4
FOUT = 32
NCORES = 8
PL = 128          # patches per core
NQ = 4            # K-chunks per patch (512 / 128)
KR = 128          # contraction rows per chunk (SBUF partitions)
NG = PL // 4      # 4-patch groups per core

_CACHE = {}


def _build_module(bufs=6, out_splits=8, mm_dtype="float32"):
    from concourse import bacc, tile, mybir

    nc = bacc.Bacc("TRN2", target_bir_lowering=False, debug=False)
    dt = mybir.dt.float32
    mdt = getattr(mybir.dt, mm_dtype)
    # xf packs data and filters: [..., 0:32] = batch cols, [..., 32:64] = fout
    xf = nc.dram_tensor("xf", [KR, PL, NQ, N + FOUT], mdt, kind="ExternalInput").ap()
    bt = nc.dram_tensor("bt", [KR, 1], dt, kind="ExternalInput").ap()
    out = nc.dram_tensor("out", [KR, NG, N], dt, kind="ExternalOutput").ap()

    # Graduated chunk sizes (in patches): small first chunks so the first
    # matmul isn't gated on a full-size load sharing bandwidth round-robin.
    sizes = [2, 2, 4]
    rest = PL - sum(sizes)
    sizes += [8] * (rest // 8)
    assert sum(sizes) == PL
    GSPLIT = NG // out_splits
    relu = mybir.ActivationFunctionType.Relu

    with tile.TileContext(nc) as tc:
        with (
            tc.tile_pool(name="xfpool", bufs=bufs) as xfpool,
            tc.tile_pool(name="psum", bufs=8, space="PSUM") as psum,
            tc.tile_pool(name="misc", bufs=1) as misc,
        ):
            bias_t = misc.tile([KR, 1], dt)
            nc.sync.dma_start(bias_t[:], bt[:])
            staging = misc.tile([KR, NG, N], dt)

            p0 = 0
            for ch, PC in enumerate(sizes):
                xtile = xfpool.tile([KR, PC, NQ, N + FOUT], mdt, tag="xf")
                sl = slice(p0, p0 + PC)
                eng = nc.sync if ch % 2 == 0 else nc.scalar
                eng.dma_start(xtile[:], xf[:, sl, :, :])
                for g in range(PC // 2):
                    gg = (p0 + g * 2) // 4       # psum group id (2 patches/iter)
                    half = (p0 + g * 2) % 4      # 0 or 2: which half of the group
                    if half == 0:
                        ptile = psum.tile([KR, N], dt, tag="ps")
                    for s2 in range(2):
                        s = half + s2
                        p = g * 2 + s2
                        for q in range(NQ):
                            nc.tensor.matmul(
                                ptile[32 * s : 32 * s + 32, :],
                                xtile[:, p, q, N : N + FOUT],  # lhsT [128,32(o)]
                                xtile[:, p, q, 0:N],           # rhs  [128,32(b)]
                                start=(q == 0),
                                stop=(q == NQ - 1),
                                tile_position=(0, 32 * s),
                            )
                    if half == 2:
                        nc.scalar.activation(
                            staging[:, gg, :], ptile[:], relu, bias=bias_t[:]
                        )
                        if (gg + 1) % GSPLIT == 0:
                            osl = slice(gg + 1 - GSPLIT, gg + 1)
                            oeng = nc.sync if gg + 1 == NG else nc.gpsimd
                            oeng.dma_start(out[:, osl, :], staging[:, osl, :])
                p0 += PC
    nc.compile()
    return nc


def _build_module_r(bufs=6):
    """float32r variant: single-pass fp32 matmuls (tf32-ish precision),
    PSUM packing along the free axis (8 patches per bank) since fp32r
    requires dst base partition 0. Half the PE instruction stream of the
    fp32 variant -> fewer IRAM paging stalls."""
    from concourse import bacc, tile, mybir

    nc = bacc.Bacc("TRN2", target_bir_lowering=False, debug=False)
    dt = mybir.dt.float32
    mdt = mybir.dt.float32r
    SG = 8                      # patches per PSUM super-group
    NSG = PL // SG              # 16
    xf = nc.dram_tensor("xf", [KR, PL, NQ, N + FOUT], mdt, kind="ExternalInput").ap()
    bt = nc.dram_tensor("bt", [FOUT, 1], dt, kind="ExternalInput").ap()
    out = nc.dram_tensor("out", [FOUT, PL, N], dt, kind="ExternalOutput").ap()

    # Graduated head (fast first matmul) and tail (short final chain).
    sizes = [2, 2, 4]
    rest = PL - sum(sizes) - 8
    sizes += [8] * (rest // 8) + [4, 2, 2]
    assert sum(sizes) == PL
    relu = mybir.ActivationFunctionType.Relu

    with tile.TileContext(nc) as tc:
        with (
            tc.tile_pool(name="xfpool", bufs=bufs) as xfpool,
            tc.tile_pool(name="psum", bufs=6, space="PSUM") as psum,
            tc.tile_pool(name="misc", bufs=1) as misc,
        ):
            bias_t = misc.tile([FOUT, 1], dt)
            nc.sync.dma_start(bias_t[:], bt[:])
            staging = misc.tile([FOUT, NSG, SG, N], dt)

            p0 = 0
            ptile = None
            for ch, PC in enumerate(sizes):
                xtile = xfpool.tile([KR, PC, NQ, N + FOUT], mdt, tag="xf")
                eng = nc.sync if ch % 2 == 0 else nc.scalar
                eng.dma_start(xtile[:], xf[:, p0 : p0 + PC, :, :])
                for pl in range(PC):
                    p = p0 + pl
                    sg, i = divmod(p, SG)
                    if i == 0:
                        ptile = psum.tile([FOUT, SG, N], dt, tag="ps")
                    for q in range(NQ):
                        nc.tensor.matmul(
                            ptile[:, i, :],
                            xtile[:, pl, q, N : N + FOUT],  # lhsT [128,32(o)]
                            xtile[:, pl, q, 0:N],           # rhs  [128,32(b)]
                            start=(q == 0),
                            stop=(q == NQ - 1),
                        )
                    if i == SG - 1:
                        nc.scalar.activation(
                            staging[:, sg, :, :], ptile[:], relu, bias=bias_t[:]
                        )
                        # store every 2 super-groups; per-sg near the end
                        if sg >= NSG - 2:
                            osl = slice(sg * SG, (sg + 1) * SG)
                            oeng = nc.sync if sg == NSG - 1 else nc.gpsimd
                            oeng.dma_start(out[:, osl, :], staging[:, sg : sg + 1, :, :])
                        elif sg % 2 == 1:
                            osl = slice((sg - 1) * SG, (sg + 1) * SG)
                            nc.gpsimd.dma_start(out[:, osl, :], staging[:, sg - 1 : sg + 1, :, :])
                p0 += PC
    nc.compile()
    return nc


def _get_module():
    if "nc" not in _CACHE:
        _CACHE["nc"] = _build_module()
    return _CACHE["nc"]


def _marshal(X, filters, bias):
    """Shard + lay out full inputs into per-core device arrays."""
    X = np.ascontiguousarray(np.asarray(X, dtype=np.float32))
    filters = np.ascontiguousarray(np.asarray(filters, dtype=np.float32))
    bias = np.asarray(bias, dtype=np.float32)

    # X: (b, core, pr, i, pc, j, c) -> (core, j, c, pr, pc, i, b)
    xv = X.reshape(N, NCORES, 4, FH, 32, FW, C)
    xt = xv.transpose(1, 5, 6, 2, 4, 3, 0).reshape(NCORES, KR, PL, NQ, N)
    # filters: (core, p, i, j, c, o) -> (core, j, c, p, i, o)
    fv = filters.reshape(NCORES, PL, FH, FW, C, FOUT)
    ft = fv.transpose(0, 3, 4, 1, 2, 5).reshape(NCORES, KR, PL, NQ, FOUT)
    xfa = np.ascontiguousarray(np.concatenate([xt, ft], axis=4))
    bt = np.ascontiguousarray(np.tile(bias, 4).reshape(KR, 1))
    return xfa, bt


def _assemble(outs):
    """Per-core out [128=(s,o), NG, N] -> full (N, 32, 32, FOUT)."""
    z = np.stack(outs)                                  # (core, (s,o), g, b)
    z = z.reshape(NCORES, 4, FOUT, NG, N)               # (core, s, o, g, b)
    z = z.transpose(4, 0, 3, 1, 2)                      # (b, core, g, s, o)
    z = z.reshape(N, NCORES, PL, FOUT)                  # p_loc = 4*g + s
    z = z.reshape(N, NCORES * 4, 32, FOUT)              # (b, pr_glob, pc, o)
    return np.ascontiguousarray(z)


def _assemble_r(outs):
    """Per-core out [FOUT, PL, N] -> full (N, 32, 32, FOUT)."""
    z = np.stack(outs)                                  # (core, o, p, b)
    z = z.transpose(3, 0, 2, 1)                         # (b, core, p, o)
    return np.ascontiguousarray(z.reshape(N, 32, 32, FOUT))


LAST_RESULT = None
VARIANT = "fp32r"


def kernel(X, filters, bias):
    global LAST_RESULT
    from concourse.bass_utils import run_bass_kernel_spmd

    if "nc" not in _CACHE:
        _CACHE["nc"] = (
            _build_module_r() if VARIANT == "fp32r" else _build_module()
        )
    nc = _CACHE["nc"]
    xfa, bt = _marshal(X, filters, bias)
    if VARIANT == "fp32r":
        bt = np.ascontiguousarray(bt[:FOUT])
    in_maps = [{"xf": xfa[k], "bt": bt} for k in range(NCORES)]
    res = run_bass_kernel_spmd(nc, in_maps, core_ids=list(range(NCORES)))
    LAST_RESULT = res
    outs = [res.results[k]["out"] for k in range(NCORES)]
    return _assemble_r(outs) if VARIANT == "fp32r" else _assemble(outs)


# revision 16
# speedup vs baseline: 1.1232x; 1.0114x over previous
"""Locally-connected conv (BioConvolution) Trainium2 kernel.

Problem: Z[n,p,o] = relu(sum_{ijc} patch[n,p,i,j,c] * filt[p,i,j,c,o] + bias[o])
  X: (32,128,128,32) f32, filters: (1024,4,4,32,32) f32, bias: (32,)
  out: (32,32,32,32) f32.   FH=FW=4 non-overlapping patches, P=1024.

Sharding: patch-parallel over P across 8 cores. Core k owns patches
[128k,128k+128) == image rows [16k,16k+16). Each core touches only its own
X rows and filters: 16.8 MB in + 0.5 MB out per core — the true memory
roofline (no operand is reused across cores).

Host-side marshaling (part of sharding): the contraction axis must sit on
SBUF partitions for the PE, so X is pre-arranged per-core into
  xt[r, p, q, b] = X[b, 16k+4*pr+q, 4*pc+j, c]   (r = j*32+c, p = pr*32+pc)
and filters into the matching
  ft[r, p, q, o] = filters[128k+p, q, j, c, o].
Both are r-major so every HBM->SBUF DMA moves long contiguous runs per
partition (multi-KB descriptors at line rate).

Device kernel (identical SPMD program on 8 cores):
  for each 4-patch group: 16 fp32 matmuls (K=128, M=32 fout, N=32 batch)
  accumulate into one PSUM [128,32] tile (col-tiled: patch s -> partitions
  32s..32s+32), ScalarE applies bias+ReLU into an SBUF staging buffer,
  one contiguous 512 KB output DMA at the end.
"""

import numpy as np

N, H, W, C = 32, 128, 128, 32
FH = FW = 4
FOUT = 32
NCORES = 8
PL = 128          # patches per core
NQ = 4            # K-chunks per patch (512 / 128)
KR = 128          # contraction rows per chunk (SBUF partitions)
NG = PL // 4      # 4-patch groups per core

_CACHE = {}


def _build_module(bufs=6, out_splits=8, mm_dtype="float32"):
    from concourse import bacc, tile, mybir

    nc = bacc.Bacc("TRN2", target_bir_lowering=False, debug=False)
    dt = mybir.dt.float32
    mdt = getattr(mybir.dt, mm_dtype)
    # xf packs data and filters: [..., 0:32] = batch cols, [..., 32:64] = fout
    xf = nc.dram_tensor("xf", [KR, PL, NQ, N + FOUT], mdt, kind="ExternalInput").ap()
    bt = nc.dram_tensor("bt", [KR, 1], dt, kind="ExternalInput").ap()
    out = nc.dram_tensor("out", [KR, NG, N], dt, kind="ExternalOutput").ap()

    # Graduated chunk sizes (in patches): small first chunks so the first
    # matmul isn't gated on a full-size load sharing bandwidth round-robin.
    sizes = [2, 2, 4]
    rest = PL - sum(sizes)
    sizes += [8] * (rest // 8)
    assert sum(sizes) == PL
    GSPLIT = NG // out_splits
    relu = mybir.ActivationFunctionType.Relu

    with tile.TileContext(nc) as tc:
        with (
            tc.tile_pool(name="xfpool", bufs=bufs) as xfpool,
            tc.tile_pool(name="psum", bufs=8, space="PSUM") as psum,
            tc.tile_pool(name="misc", bufs=1) as misc,
        ):
            bias_t = misc.tile([KR, 1], dt)
            nc.sync.dma_start(bias_t[:], bt[:])
            staging = misc.tile([KR, NG, N], dt)

            p0 = 0
            for ch, PC in enumerate(sizes):
                xtile = xfpool.tile([KR, PC, NQ, N + FOUT], mdt, tag="xf")
                sl = slice(p0, p0 + PC)
                eng = nc.sync if ch % 2 == 0 else nc.scalar
                eng.dma_start(xtile[:], xf[:, sl, :, :])
                for g in range(PC // 2):
                    gg = (p0 + g * 2) // 4       # psum group id (2 patches/iter)
                    half = (p0 + g * 2) % 4      # 0 or 2: which half of the group
                    if half == 0:
                        ptile = psum.tile([KR, N], dt, tag="ps")
                    for s2 in range(2):
                        s = half + s2
                        p = g * 2 + s2
                        for q in range(NQ):
                            nc.tensor.matmul(
                                ptile[32 * s : 32 * s + 32, :],
                                xtile[:, p, q, N : N + FOUT],  # lhsT [128,32(o)]
                                xtile[:, p, q, 0:N],           # rhs  [128,32(b)]
                                start=(q == 0),
                                stop=(q == NQ - 1),
                                tile_position=(0, 32 * s),
                            )
                    if half == 2:
                        nc.scalar.activation(
                            staging[:, gg, :], ptile[:], relu, bias=bias_t[:]
                        )
                        if (gg + 1) % GSPLIT == 0:
                            osl = slice(gg + 1 - GSPLIT, gg + 1)
                            oeng = nc.sync if gg + 1 == NG else nc.gpsimd
                            oeng.dma_start(out[:, osl, :], staging[:, osl, :])
                p0 += PC
    nc.compile()
    return nc


def _build_module_r(bufs=6):
    """float32r variant: single-pass fp32 matmuls (tf32-ish precision),
    PSUM packing along the free axis (8 patches per bank) since fp32r
    requires dst base partition 0. Half the PE instruction stream of the
    fp32 variant -> fewer IRAM paging stalls."""
    from concourse import bacc, tile, mybir

    nc = bacc.Bacc("TRN2", target_bir_lowering=False, debug=False)
    dt = mybir.dt.float32
    mdt = mybir.dt.float32r
    SG = 8                      # patches per PSUM super-group
    NSG = PL // SG              # 16
    xf = nc.dram_tensor("xf", [KR, PL, NQ, N + FOUT], mdt, kind="ExternalInput").ap()
    bt = nc.dram_tensor("bt", [FOUT, 1], dt, kind="ExternalInput").ap()
    out = nc.dram_tensor("out", [FOUT, PL, N], dt, kind="ExternalOutput").ap()

    # Graduated head (fast first matmul) and tail (short final chain).
    sizes = [2, 2, 4]
    rest = PL - sum(sizes) - 8
    sizes += [8] * (rest // 8) + [4, 2, 2]
    assert sum(sizes) == PL
    relu = mybir.ActivationFunctionType.Relu

    with tile.TileContext(nc) as tc:
        with (
            tc.tile_pool(name="xfpool", bufs=bufs) as xfpool,
            tc.tile_pool(name="psum", bufs=6, space="PSUM") as psum,
            tc.tile_pool(name="misc", bufs=1) as misc,
        ):
            bias_t = misc.tile([FOUT, 1], dt)
            nc.sync.dma_start(bias_t[:], bt[:])
            staging = misc.tile([FOUT, NSG, SG, N], dt)

            p0 = 0
            ptile = None
            for ch, PC in enumerate(sizes):
                xtile = xfpool.tile([KR, PC, NQ, N + FOUT], mdt, tag="xf")
                # Single issuing engine => single HWDGE FIFO => loads complete
                # strictly in order (no round-robin synchronized completions).
                nc.sync.dma_start(xtile[:], xf[:, p0 : p0 + PC, :, :])
                for pl in range(PC):
                    p = p0 + pl
                    sg, i = divmod(p, SG)
                    if i == 0:
                        ptile = psum.tile([FOUT, SG, N], dt, tag="ps")
                    for q in range(NQ):
                        nc.tensor.matmul(
                            ptile[:, i, :],
                            xtile[:, pl, q, N : N + FOUT],  # lhsT [128,32(o)]
                            xtile[:, pl, q, 0:N],           # rhs  [128,32(b)]
                            start=(q == 0),
                            stop=(q == NQ - 1),
                        )
                    if i == SG - 1:
                        nc.scalar.activation(
                            staging[:, sg, :, :], ptile[:], relu, bias=bias_t[:]
                        )
                        # store every 2 super-groups; per-sg near the end
                        if sg >= NSG - 2:
                            osl = slice(sg * SG, (sg + 1) * SG)
                            oeng = nc.sync if sg == NSG - 1 else nc.gpsimd
                            oeng.dma_start(out[:, osl, :], staging[:, sg : sg + 1, :, :])
                        elif sg % 2 == 1:
                            osl = slice((sg - 1) * SG, (sg + 1) * SG)
                            nc.gpsimd.dma_start(out[:, osl, :], staging[:, sg - 1 : sg + 1, :, :])
                p0 += PC
    nc.compile()
    return nc


def _get_module():
    if "nc" not in _CACHE:
        _CACHE["nc"] = _build_module()
    return _CACHE["nc"]


def _marshal(X, filters, bias):
    """Shard + lay out full inputs into per-core device arrays."""
    X = np.ascontiguousarray(np.asarray(X, dtype=np.float32))
    filters = np.ascontiguousarray(np.asarray(filters, dtype=np.float32))
    bias = np.asarray(bias, dtype=np.float32)

    # X: (b, core, pr, i, pc, j, c) -> (core, j, c, pr, pc, i, b)
    xv = X.reshape(N, NCORES, 4, FH, 32, FW, C)
    xt = xv.transpose(1, 5, 6, 2, 4, 3, 0).reshape(NCORES, KR, PL, NQ, N)
    # filters: (core, p, i, j, c, o) -> (core, j, c, p, i, o)
    fv = filters.reshape(NCORES, PL, FH, FW, C, FOUT)
    ft = fv.transpose(0, 3, 4, 1, 2, 5).reshape(NCORES, KR, PL, NQ, FOUT)
    xfa = np.ascontiguousarray(np.concatenate([xt, ft], axis=4))
    bt = np.ascontiguousarray(np.tile(bias, 4).reshape(KR, 1))
    return xfa, bt


def _assemble(outs):
    """Per-core out [128=(s,o), NG, N] -> full (N, 32, 32, FOUT)."""
    z = np.stack(outs)                                  # (core, (s,o), g, b)
    z = z.reshape(NCORES, 4, FOUT, NG, N)               # (core, s, o, g, b)
    z = z.transpose(4, 0, 3, 1, 2)                      # (b, core, g, s, o)
    z = z.reshape(N, NCORES, PL, FOUT)                  # p_loc = 4*g + s
    z = z.reshape(N, NCORES * 4, 32, FOUT)              # (b, pr_glob, pc, o)
    return np.ascontiguousarray(z)


def _assemble_r(outs):
    """Per-core out [FOUT, PL, N] -> full (N, 32, 32, FOUT)."""
    z = np.stack(outs)                                  # (core, o, p, b)
    z = z.transpose(3, 0, 2, 1)                         # (b, core, p, o)
    return np.ascontiguousarray(z.reshape(N, 32, 32, FOUT))


LAST_RESULT = None
VARIANT = "fp32r"


def kernel(X, filters, bias):
    global LAST_RESULT
    from concourse.bass_utils import run_bass_kernel_spmd

    if "nc" not in _CACHE:
        _CACHE["nc"] = (
            _build_module_r() if VARIANT == "fp32r" else _build_module()
        )
    nc = _CACHE["nc"]
    xfa, bt = _marshal(X, filters, bias)
    if VARIANT == "fp32r":
        bt = np.ascontiguousarray(bt[:FOUT])
    in_maps = [{"xf": xfa[k], "bt": bt} for k in range(NCORES)]
    res = run_bass_kernel_spmd(nc, in_maps, core_ids=list(range(NCORES)))
    LAST_RESULT = res
    outs = [res.results[k]["out"] for k in range(NCORES)]
    return _assemble_r(outs) if VARIANT == "fp32r" else _assemble(outs)


# revision 18
# speedup vs baseline: 1.1653x; 1.0375x over previous
"""Locally-connected conv (BioConvolution) Trainium2 kernel.

Problem: Z[n,p,o] = relu(sum_{ijc} patch[n,p,i,j,c] * filt[p,i,j,c,o] + bias[o])
  X: (32,128,128,32) f32, filters: (1024,4,4,32,32) f32, bias: (32,)
  out: (32,32,32,32) f32.   FH=FW=4 non-overlapping patches, P=1024.

Sharding: patch-parallel over P across 8 cores. Core k owns patches
[128k,128k+128) == image rows [16k,16k+16). Each core touches only its own
X rows and filters: 16.8 MB in + 0.5 MB out per core — the true memory
roofline (no operand is reused across cores).

Host-side marshaling (part of sharding): the contraction axis must sit on
SBUF partitions for the PE, so X is pre-arranged per-core into
  xt[r, p, q, b] = X[b, 16k+4*pr+q, 4*pc+j, c]   (r = j*32+c, p = pr*32+pc)
and filters into the matching
  ft[r, p, q, o] = filters[128k+p, q, j, c, o].
Both are r-major so every HBM->SBUF DMA moves long contiguous runs per
partition (multi-KB descriptors at line rate).

Device kernel (identical SPMD program on 8 cores):
  for each 4-patch group: 16 fp32 matmuls (K=128, M=32 fout, N=32 batch)
  accumulate into one PSUM [128,32] tile (col-tiled: patch s -> partitions
  32s..32s+32), ScalarE applies bias+ReLU into an SBUF staging buffer,
  one contiguous 512 KB output DMA at the end.
"""

import numpy as np

N, H, W, C = 32, 128, 128, 32
FH = FW = 4
FOUT = 32
NCORES = 8
PL = 128          # patches per core
NQ = 4            # K-chunks per patch (512 / 128)
KR = 128          # contraction rows per chunk (SBUF partitions)
NG = PL // 4      # 4-patch groups per core

_CACHE = {}


def _build_module(bufs=6, out_splits=8, mm_dtype="float32"):
    from concourse import bacc, tile, mybir

    nc = bacc.Bacc("TRN2", target_bir_lowering=False, debug=False)
    dt = mybir.dt.float32
    mdt = getattr(mybir.dt, mm_dtype)
    # xf packs data and filters: [..., 0:32] = batch cols, [..., 32:64] = fout
    xf = nc.dram_tensor("xf", [KR, PL, NQ, N + FOUT], mdt, kind="ExternalInput").ap()
    bt = nc.dram_tensor("bt", [KR, 1], dt, kind="ExternalInput").ap()
    out = nc.dram_tensor("out", [KR, NG, N], dt, kind="ExternalOutput").ap()

    # Graduated chunk sizes (in patches): small first chunks so the first
    # matmul isn't gated on a full-size load sharing bandwidth round-robin.
    sizes = [2, 2, 4]
    rest = PL - sum(sizes)
    sizes += [8] * (rest // 8)
    assert sum(sizes) == PL
    GSPLIT = NG // out_splits
    relu = mybir.ActivationFunctionType.Relu

    with tile.TileContext(nc) as tc:
        with (
            tc.tile_pool(name="xfpool", bufs=bufs) as xfpool,
            tc.tile_pool(name="psum", bufs=8, space="PSUM") as psum,
            tc.tile_pool(name="misc", bufs=1) as misc,
        ):
            bias_t = misc.tile([KR, 1], dt)
            nc.sync.dma_start(bias_t[:], bt[:])
            staging = misc.tile([KR, NG, N], dt)

            p0 = 0
            for ch, PC in enumerate(sizes):
                xtile = xfpool.tile([KR, PC, NQ, N + FOUT], mdt, tag="xf")
                sl = slice(p0, p0 + PC)
                eng = nc.sync if ch % 2 == 0 else nc.scalar
                eng.dma_start(xtile[:], xf[:, sl, :, :])
                for g in range(PC // 2):
                    gg = (p0 + g * 2) // 4       # psum group id (2 patches/iter)
                    half = (p0 + g * 2) % 4      # 0 or 2: which half of the group
                    if half == 0:
                        ptile = psum.tile([KR, N], dt, tag="ps")
                    for s2 in range(2):
                        s = half + s2
                        p = g * 2 + s2
                        for q in range(NQ):
                            nc.tensor.matmul(
                                ptile[32 * s : 32 * s + 32, :],
                                xtile[:, p, q, N : N + FOUT],  # lhsT [128,32(o)]
                                xtile[:, p, q, 0:N],           # rhs  [128,32(b)]
                                start=(q == 0),
                                stop=(q == NQ - 1),
                                tile_position=(0, 32 * s),
                            )
                    if half == 2:
                        nc.scalar.activation(
                            staging[:, gg, :], ptile[:], relu, bias=bias_t[:]
                        )
                        if (gg + 1) % GSPLIT == 0:
                            osl = slice(gg + 1 - GSPLIT, gg + 1)
                            oeng = nc.sync if gg + 1 == NG else nc.gpsimd
                            oeng.dma_start(out[:, osl, :], staging[:, osl, :])
                p0 += PC
    nc.compile()
    return nc


def _build_module_r(bufs=6):
    """float32r variant: single-pass fp32 matmuls (tf32-ish precision),
    PSUM packing along the free axis (8 patches per bank) since fp32r
    requires dst base partition 0. Half the PE instruction stream of the
    fp32 variant -> fewer IRAM paging stalls."""
    from concourse import bacc, tile, mybir

    nc = bacc.Bacc("TRN2", target_bir_lowering=False, debug=False)
    dt = mybir.dt.float32
    mdt = mybir.dt.float32r
    SG = 8                      # patches per PSUM super-group
    NSG = PL // SG              # 16
    xf = nc.dram_tensor("xf", [KR, PL, NQ, N + FOUT], mdt, kind="ExternalInput").ap()
    bt = nc.dram_tensor("bt", [FOUT, 1], dt, kind="ExternalInput").ap()
    out = nc.dram_tensor("out", [FOUT, PL, N], dt, kind="ExternalOutput").ap()

    # Graduated head: small first chunks so the first matmul starts early.
    sizes = [2, 2, 4]
    rest = PL - sum(sizes)
    sizes += [8] * (rest // 8)
    assert sum(sizes) == PL
    relu = mybir.ActivationFunctionType.Relu

    with tile.TileContext(nc) as tc:
        with (
            tc.tile_pool(name="xfpool", bufs=bufs) as xfpool,
            tc.tile_pool(name="psum", bufs=6, space="PSUM") as psum,
            tc.tile_pool(name="misc", bufs=1) as misc,
        ):
            bias_t = misc.tile([FOUT, 1], dt)
            nc.sync.dma_start(bias_t[:], bt[:])
            staging = misc.tile([FOUT, NSG, SG, N], dt)

            p0 = 0
            ptile = None
            for ch, PC in enumerate(sizes):
                xtile = xfpool.tile([KR, PC, NQ, N + FOUT], mdt, tag="xf")
                # Single issuing engine => single HWDGE FIFO => loads complete
                # strictly in order (no round-robin synchronized completions).
                nc.sync.dma_start(xtile[:], xf[:, p0 : p0 + PC, :, :])
                for pl in range(PC):
                    p = p0 + pl
                    sg, i = divmod(p, SG)
                    if i == 0:
                        ptile = psum.tile([FOUT, SG, N], dt, tag="ps")
                    for q in range(NQ):
                        nc.tensor.matmul(
                            ptile[:, i, :],
                            xtile[:, pl, q, N : N + FOUT],  # lhsT [128,32(o)]
                            xtile[:, pl, q, 0:N],           # rhs  [128,32(b)]
                            start=(q == 0),
                            stop=(q == NQ - 1),
                        )
                    if i == SG - 1:
                        nc.scalar.activation(
                            staging[:, sg, :, :], ptile[:], relu, bias=bias_t[:]
                        )
                        # store every 2 super-groups (gpsimd SWDGE, off the
                        # load ring); final pair goes on sync's FIFO so it
                        # starts the moment the last ACT lands.
                        if sg % 2 == 1:
                            osl = slice((sg - 1) * SG, (sg + 1) * SG)
                            oeng = nc.sync if sg == NSG - 1 else nc.gpsimd
                            oeng.dma_start(out[:, osl, :], staging[:, sg - 1 : sg + 1, :, :])
                p0 += PC
    nc.compile()
    return nc


def _get_module():
    if "nc" not in _CACHE:
        _CACHE["nc"] = _build_module()
    return _CACHE["nc"]


def _marshal(X, filters, bias):
    """Shard + lay out full inputs into per-core device arrays."""
    X = np.ascontiguousarray(np.asarray(X, dtype=np.float32))
    filters = np.ascontiguousarray(np.asarray(filters, dtype=np.float32))
    bias = np.asarray(bias, dtype=np.float32)

    # X: (b, core, pr, i, pc, j, c) -> (core, j, c, pr, pc, i, b)
    xv = X.reshape(N, NCORES, 4, FH, 32, FW, C)
    xt = xv.transpose(1, 5, 6, 2, 4, 3, 0).reshape(NCORES, KR, PL, NQ, N)
    # filters: (core, p, i, j, c, o) -> (core, j, c, p, i, o)
    fv = filters.reshape(NCORES, PL, FH, FW, C, FOUT)
    ft = fv.transpose(0, 3, 4, 1, 2, 5).reshape(NCORES, KR, PL, NQ, FOUT)
    xfa = np.ascontiguousarray(np.concatenate([xt, ft], axis=4))
    bt = np.ascontiguousarray(np.tile(bias, 4).reshape(KR, 1))
    return xfa, bt


def _assemble(outs):
    """Per-core out [128=(s,o), NG, N] -> full (N, 32, 32, FOUT)."""
    z = np.stack(outs)                                  # (core, (s,o), g, b)
    z = z.reshape(NCORES, 4, FOUT, NG, N)               # (core, s, o, g, b)
    z = z.transpose(4, 0, 3, 1, 2)                      # (b, core, g, s, o)
    z = z.reshape(N, NCORES, PL, FOUT)                  # p_loc = 4*g + s
    z = z.reshape(N, NCORES * 4, 32, FOUT)              # (b, pr_glob, pc, o)
    return np.ascontiguousarray(z)


def _assemble_r(outs):
    """Per-core out [FOUT, PL, N] -> full (N, 32, 32, FOUT)."""
    z = np.stack(outs)                                  # (core, o, p, b)
    z = z.transpose(3, 0, 2, 1)                         # (b, core, p, o)
    return np.ascontiguousarray(z.reshape(N, 32, 32, FOUT))


LAST_RESULT = None
VARIANT = "fp32r"


def kernel(X, filters, bias):
    global LAST_RESULT
    from concourse.bass_utils import run_bass_kernel_spmd

    if "nc" not in _CACHE:
        _CACHE["nc"] = (
            _build_module_r() if VARIANT == "fp32r" else _build_module()
        )
    nc = _CACHE["nc"]
    xfa, bt = _marshal(X, filters, bias)
    if VARIANT == "fp32r":
        bt = np.ascontiguousarray(bt[:FOUT])
    in_maps = [{"xf": xfa[k], "bt": bt} for k in range(NCORES)]
    res = run_bass_kernel_spmd(nc, in_maps, core_ids=list(range(NCORES)))
    LAST_RESULT = res
    outs = [res.results[k]["out"] for k in range(NCORES)]
    return _assemble_r(outs) if VARIANT == "fp32r" else _assemble(outs)
